# revision 1
# baseline (speedup 1.0000x reference)
"""GroupedMLP (MoE, uniform routing) on 8 NeuronCores via expert parallelism.

Math (per expert e, tokens pre-sorted):
    gate_up = x_e @ w_gate_up[e].T        # [2048, 8192]
    hidden  = silu(gate) * up             # [2048, 4096]
    out_e   = hidden @ w_down[e].T        # [2048, 2048]

Core i handles expert i (T/E = 2048 tokens).  Everything is computed in the
transposed domain so both matmul operands have the contraction dim on the
SBUF partition axis with zero on-chip transposes:
    gate_upT[f, t] = sum_h w_guT[h, f] * xT[h, t]
    outT[h', t]    = sum_f w_dnT[f, h'] * hidT[f, t]
Host packs x/w into bf16 tiles laid out exactly as SBUF wants them; PSUM
accumulates in fp32.
"""

import numpy as np
import ml_dtypes

E, H, F, T = 8, 2048, 4096, 16384
TPE = T // E          # 2048 tokens per expert/core
NCORES = 8
TB = 512              # token block (PSUM bank width in fp32)
NTB = TPE // TB       # 4
NHO = H // 128        # 16 contraction blocks for MM1
NFB = 2 * F // 128    # 64 f-blocks of gate_up (32 gate + 32 up)
NFO = F // 128        # 32 contraction blocks for MM2
NHB = H // 128        # 16 output h'-blocks

BF16 = ml_dtypes.bfloat16

_CACHE = {}


def _split_multiwaits(nc, mybir, bass_rust):
    """walrus CTRL-format instructions on this compiler accept only one sem
    wait; hoist extra waits onto single-wait NOPs spliced just before."""
    for f in nc.m.functions:
        for b in f.blocks:
            new_insts = []
            for inst in b.instructions:
                si = inst.sync_info
                if si is not None and si.on_wait and len(si.on_wait) > 1:
                    waits = list(si.on_wait)
                    for w in waits[:-1]:
                        nop = mybir.InstNoOp(
                            name=f"I-waitsplit-{nc.next_id()}", ins=[], outs=[]
                        )
                        nop.engine = inst.engine
                        nop.sync_info = bass_rust.SyncInfo(on_wait=[w], on_update=[])
                        new_insts.append(nop)
                    si.on_wait = [waits[-1]]
                new_insts.append(inst)
            b.instructions[:] = new_insts


def _build():
    import concourse.bass as bass
    import concourse.mybir as mybir
    import concourse.tile as tile
    import bass_rust
    from concourse.bass import ts

    nc = bass.Bass("TRN2", target_bir_lowering=False, debug=False)
    xt = nc.dram_tensor("xt", [128, NHO, TPE], mybir.dt.bfloat16, kind="ExternalInput")
    wgu = nc.dram_tensor(
        "wgu", [NFB, 128, NHO, 128], mybir.dt.bfloat16, kind="ExternalInput"
    )
    wdn = nc.dram_tensor(
        "wdn", [NHB, 128, NFO, 128], mybir.dt.bfloat16, kind="ExternalInput"
    )
    outT = nc.dram_tensor("outT", [H, TPE], mybir.dt.float32, kind="ExternalOutput")

    with tile.TileContext(nc) as tc:
        with (
            tc.tile_pool(name="xtp", bufs=1) as xt_pool,
            tc.tile_pool(name="wgup", bufs=4) as wgu_pool,
            tc.tile_pool(name="wdnp", bufs=3) as wdn_pool,
            tc.tile_pool(name="hidp", bufs=2) as hid_pool,
            tc.tile_pool(name="tmpp", bufs=4) as tmp_pool,
            tc.tile_pool(name="obp", bufs=4) as ob_pool,
            tc.tile_pool(name="pgu", bufs=6, space="PSUM") as pgu_pool,
            tc.tile_pool(name="po", bufs=2, space="PSUM") as po_pool,
        ):
            xt_t = xt_pool.tile([128, NHO, TPE], mybir.dt.bfloat16)
            nc.sync.dma_start(xt_t[:], xt[:])

            for tb in range(NTB):
                hid = hid_pool.tile([128, NFO, TB], mybir.dt.bfloat16, tag="hid")
                for jp in range(NFO):  # paired gate-(jp) / up-(jp+32) blocks
                    wg = wgu_pool.tile([128, NHO, 128], mybir.dt.bfloat16, tag="wgu")
                    nc.sync.dma_start(wg[:], wgu[jp])
                    wu = wgu_pool.tile([128, NHO, 128], mybir.dt.bfloat16, tag="wgu")
                    nc.sync.dma_start(wu[:], wgu[jp + NFO])
                    pg = pgu_pool.tile([128, TB], mybir.dt.float32, tag="pgu")
                    pu = pgu_pool.tile([128, TB], mybir.dt.float32, tag="pgu")
                    for ho in range(NHO):
                        nc.tensor.matmul(
                            pg[:],
                            wg[:, ho, :],
                            xt_t[:, ho, ts(tb, TB)],
                            start=(ho == 0),
                            stop=(ho == NHO - 1),
                        )
                    for ho in range(NHO):
                        nc.tensor.matmul(
                            pu[:],
                            wu[:, ho, :],
                            xt_t[:, ho, ts(tb, TB)],
                            start=(ho == 0),
                            stop=(ho == NHO - 1),
                        )
                    tmp = tmp_pool.tile([128, TB], mybir.dt.float32, tag="tmp")
                    nc.scalar.activation(
                        tmp[:], pg[:], mybir.ActivationFunctionType.Silu
                    )
                    nc.vector.tensor_mul(hid[:, jp, :], tmp[:], pu[:])

                for hb in range(NHB):
                    wd = wdn_pool.tile([128, NFO, 128], mybir.dt.bfloat16, tag="wdn")
                    nc.sync.dma_start(wd[:], wdn[hb])
                    po = po_pool.tile([128, TB], mybir.dt.float32, tag="po")
                    for fo in range(NFO):
                        nc.tensor.matmul(
                            po[:],
                            wd[:, fo, :],
                            hid[:, fo, :],
                            start=(fo == 0),
                            stop=(fo == NFO - 1),
                        )
                    ob = ob_pool.tile([128, TB], mybir.dt.float32, tag="ob")
                    nc.vector.tensor_copy(ob[:], po[:])
                    nc.sync.dma_start(
                        outT[ts(hb, 128), ts(tb, TB)], ob[:]
                    )

    _split_multiwaits(nc, mybir, bass_rust)
    return nc


def _get_nc():
    if "nc" not in _CACHE:
        _CACHE["nc"] = _build()
    return _CACHE["nc"]


def _pack_inputs(x, w_gate_up, w_down):
    """Per-core bf16 tile-layout packing (all layouts match SBUF exactly)."""
    x = np.asarray(x, dtype=np.float32)
    w_gate_up = np.asarray(w_gate_up, dtype=np.float32)
    w_down = np.asarray(w_down, dtype=np.float32)
    in_maps = []
    for e in range(NCORES):
        xe = x[e * TPE : (e + 1) * TPE].astype(BF16)        # [t, h]
        xt = np.ascontiguousarray(
            xe.reshape(TPE, NHO, 128).transpose(2, 1, 0)
        )                                                    # [hi, ho, t]
        wgu_e = w_gate_up[e].astype(BF16)                    # [2F, H]
        wgu_dev = np.ascontiguousarray(
            wgu_e.reshape(NFB, 128, NHO, 128).transpose(0, 3, 2, 1)
        )                                                    # [j, hi, ho, f]
        wdn_e = w_down[e].astype(BF16)                       # [H, F]
        wdn_dev = np.ascontiguousarray(
            wdn_e.reshape(NHB, 128, NFO, 128).transpose(0, 3, 2, 1)
        )                                                    # [b, fi, fo, hh]
        in_maps.append({"xt": xt, "wgu": wgu_dev, "wdn": wdn_dev})
    return in_maps


def _fixed_run_bass_via_pjrt(nc, in_maps, n_cores):
    """run_bass_via_pjrt with explicitly device-placed shards.

    The stock version passes host numpy globals into a shard_map'd jit; the
    axon PJRT backend then materializes each device shard via a compiled
    jit_dynamic_slice program, which the stock neuronx-cc takes >25 min to
    compile for our ~270 MB weight arrays.  Building the global arrays from
    per-device buffers (plain H2D copies) avoids any resharding program.
    """
    import jax
    import numpy as np
    from jax.sharding import Mesh, NamedSharding, PartitionSpec
    from jax.experimental.shard_map import shard_map
    import concourse.mybir as mybir
    from concourse import bass2jax

    bass2jax.install_neuronx_cc_hook()
    assert nc.dbg_addr is None
    partition_name = nc.partition_id_tensor.name if nc.partition_id_tensor else None

    in_names, out_names, out_avals, zero_outs = [], [], [], []
    for alloc in nc.m.functions[0].allocations:
        if not isinstance(alloc, mybir.MemoryLocationSet):
            continue
        name = alloc.memorylocations[0].name
        if alloc.kind == "ExternalInput":
            if name != partition_name:
                in_names.append(name)
        elif alloc.kind == "ExternalOutput":
            shape = tuple(alloc.tensor_shape)
            dtype = mybir.dt.np(alloc.dtype)
            out_names.append(name)
            out_avals.append(jax.core.ShapedArray(shape, dtype))
            zero_outs.append(np.zeros(shape, dtype))
    n_params = len(in_names)
    n_outs = len(out_avals)
    in_names.extend(out_names)
    if partition_name is not None:
        in_names.append(partition_name)

    donate = tuple(range(n_params, n_params + n_outs))

    def _body(*args):
        operands = list(args)
        if partition_name is not None:
            operands.append(bass2jax.partition_id_tensor())
        outs = bass2jax._bass_exec_p.bind(
            *operands,
            out_avals=tuple(out_avals),
            in_names=tuple(in_names),
            out_names=tuple(out_names),
            lowering_input_output_aliases=(),
            sim_require_finite=True,
            sim_require_nnan=True,
            nc=nc,
        )
        return tuple(outs)

    devices = jax.devices()[:n_cores]
    mesh = Mesh(np.asarray(devices), ("core",))
    sharding = NamedSharding(mesh, PartitionSpec("core"))
    in_specs = (PartitionSpec("core"),) * (n_params + n_outs)
    out_specs = (PartitionSpec("core"),) * n_outs
    sharded = jax.jit(
        shard_map(
            _body, mesh=mesh, in_specs=in_specs, out_specs=out_specs, check_rep=False
        ),
        donate_argnums=donate,
        keep_unused=True,
    )

    def _make_global(per_core_arrays):
        shape0 = per_core_arrays[0].shape
        gshape = (n_cores * shape0[0], *shape0[1:])
        shards = [
            jax.device_put(per_core_arrays[c], devices[c]) for c in range(n_cores)
        ]
        return jax.make_array_from_single_device_arrays(gshape, sharding, shards)

    global_in = [
        _make_global([np.asarray(m[in_names[i]]) for m in in_maps])
        for i in range(n_params)
    ]
    global_zero = [_make_global([z] * n_cores) for z in zero_outs]

    out_arrs = sharded(*global_in, *global_zero)
    return [
        {
            name: np.asarray(out_arrs[i]).reshape(n_cores, *out_avals[i].shape)[c]
            for i, name in enumerate(out_names)
        }
        for c in range(n_cores)
    ]


def kernel(x, w_gate_up, w_down, tokens_per_expert, _trace=False):
    from concourse import bass2jax
    from concourse.bass_utils import run_bass_kernel_spmd

    bass2jax.run_bass_via_pjrt = _fixed_run_bass_via_pjrt
    nc = _get_nc()
    in_maps = _pack_inputs(x, w_gate_up, w_down)
    res = run_bass_kernel_spmd(
        nc, in_maps, core_ids=list(range(NCORES)), trace=_trace
    )
    _CACHE["last_result"] = res
    out = np.empty((T, H), dtype=np.float32)
    for e in range(NCORES):
        out[e * TPE : (e + 1) * TPE] = res.results[e]["outT"].T
    return out

